# revision 38
# baseline (speedup 1.0000x reference)
"""Multi-head attention kernel for Trainium2, 8 NeuronCores, data-parallel over batch.

Problem: batch=16, pos=577, d_model=1024, n_heads=16, d_head=64, fp32.
Sharding: batch across 8 cores (2 batch items per core), no collectives.

v5 (final, ~268us vs 310us baseline): phase B restructured around big exp
instructions and merged AV matmuls.
  - q chunks (512, 65): one S stationary per (kt, hd) serves both chunks;
    exp instructions are [128,512] (q0) plus one batched strided exp per hd
    covering all five kt's 65-wide q1 slots, emitted as soon as kt4's S
    lands so the round tail is short.
  - AV stationary is [ones64 | V_h] (M=128, contiguous, FWL-eligible):
    PSUM rows 0:64 = softmax denominator replicated 64x, rows 64:128 = Z'.
    Normalization = 64-lane reciprocal_approx_fast (base-0 only!) +
    tensor-tensor multiply straight out of PSUM.
  - b_V folded into the V projection eviction (Z'/D = PV/D + b_V exactly);
    b_Q/b_K fused in Q/K evicts; b_O fused in the C-phase evict.
  - Tail: emit_attn(0,7) -> C(b=0) -> emit_attn(1,7) -> C(b=1) keeps the PE
    warm through the B->C transition.

PSUM banks (8): psS 2 (rotating S q0 staging), pq1a 1 (S-q1 slots hd0),
pq1b 1 (S-q1 slots hd1 + both AV-q1 chains, sequential), psZ 2 (AV-q0
accumulators per hd), psA 2 (Q/K projection staging).

has_written semantics learned the hard way: accumulation chains sharing a
bank must not interleave their start=True openers; sequential chains and
write-once groups are safe.
"""
import numpy as np

import concourse.bass as bass
import concourse.tile as tile
from concourse import bacc, mybir

F32 = mybir.dt.float32
BF16 = mybir.dt.bfloat16
AF = mybir.ActivationFunctionType

NCORES = 8
_DEBUG_DUMPS = None
B = 2            # batch per core
T = 577
D = 1024
H = 16
E = 64
HE = H * E       # 1024
BT = B * T       # 1154

KT = 8                                   # k-tiles over D
MT = 8                                   # m-tiles over HE (head pairs)
A_N = [(0, 386), (386, 384), (770, 384)]  # bt chunks for phase A
TT = [(0, 128), (128, 128), (256, 128), (384, 128), (512, 65)]  # tiles over T
N512 = [(0, 512), (512, 512)]            # 512-chunks over HE / D
VGW = H * 128                            # 2048: per (b,tile) [ones|V] groups
Q0 = 512                                 # q0 chunk width
Q1 = T - Q0                              # 65: q1 chunk width


def build_graph():
    nc = bacc.Bacc("TRN2", target_bir_lowering=False, debug=False,
                   num_devices=NCORES)

    xq = nc.dram_tensor("query_input", [D, BT], BF16, kind="ExternalInput")
    xk = nc.dram_tensor("key_input", [D, BT], BF16, kind="ExternalInput")
    xv = nc.dram_tensor("value_input", [D, BT], BF16, kind="ExternalInput")
    wq = nc.dram_tensor("W_Q", [D, HE], BF16, kind="ExternalInput")
    wk = nc.dram_tensor("W_K", [D, HE], BF16, kind="ExternalInput")
    wv = nc.dram_tensor("W_V", [D, HE], BF16, kind="ExternalInput")
    wo = nc.dram_tensor("W_O", [HE, D], BF16, kind="ExternalInput")
    bq = nc.dram_tensor("b_Q", [128, MT], F32, kind="ExternalInput")
    bk = nc.dram_tensor("b_K", [128, MT], F32, kind="ExternalInput")
    bv = nc.dram_tensor("b_V", [1, HE], BF16, kind="ExternalInput")
    bo = nc.dram_tensor("b_O", [1, D], BF16, kind="ExternalInput")
    out = nc.dram_tensor("out", [B, T, D], F32, kind="ExternalOutput")

    with tile.TileContext(nc) as tc:
        _body(nc, tc, xq, xk, xv, wq, wk, wv, wo, bq, bk, bv, bo, out)
    nc.compile()
    return nc


def _body(nc, tc, xq, xk, xv, wq, wk, wv, wo, bq, bk, bv, bo, out):
    from contextlib import ExitStack
    _last_pp = [None]
    est = ExitStack()
    with est:
        # ---- persistent pools; packed tiles ----
        sbQ_p = est.enter_context(tc.tile_pool(name="sbQ", bufs=1))
        sbK_p = est.enter_context(tc.tile_pool(name="sbK", bufs=1))
        sbVg_p = est.enter_context(tc.tile_pool(name="sbVg", bufs=1))
        sbZ_p = est.enter_context(tc.tile_pool(name="sbZ", bufs=1))
        xt_p = est.enter_context(tc.tile_pool(name="xt", bufs=2))
        wt_p = est.enter_context(tc.tile_pool(name="wt", bufs=3))
        const_p = est.enter_context(tc.tile_pool(name="const", bufs=1))

        bqc = const_p.tile([128, MT], F32, tag="bqc")
        bkc = const_p.tile([128, MT], F32, tag="bkc")
        bvb = const_p.tile([128, HE], BF16, tag="bvb")
        boc = const_p.tile([128, D], BF16, tag="boc")

        # packed persistent tiles (bf16)
        sbQ = sbQ_p.tile([128, MT * BT], BF16, tag="sbQ")     # [:, m*BT + bt]
        sbK = sbK_p.tile([128, MT * BT], BF16, tag="sbK")
        # per (b,tile): 16 head-groups of [ones64 | V_h 64]
        sbVg = sbVg_p.tile([128, 10 * VGW], BF16, tag="sbVg")
        sbZ = sbZ_p.tile([128, B * MT * T], BF16, tag="sbZ")  # [:, (b*MT+hp)*T + t]

        def zsl(b, hp, lo, sz, to, tsz):
            base = (b * MT + hp) * T
            return sbZ[lo:lo + sz, base + to:base + to + tsz]

        # ================= Phase A: projections =================
        def load_xw(x_in, w_in, pool, split_k0=False):
            xt = pool.tile([128, KT * BT], BF16, tag="xt", name="xt")
            wt = wt_p.tile([128, KT * HE], BF16, tag="wt", name="wt")
            for k in range(KT):
                if k == 0 and split_k0:
                    for c0, c1 in ((0, 128), (128, BT)):
                        nc.sync.dma_start(xt[:, c0:c1],
                                          x_in.ap()[0:128, c0:c1])
                    for c0, c1 in ((0, 256), (256, 512), (512, HE)):
                        nc.sync.dma_start(wt[:, c0:c1],
                                          w_in.ap()[0:128, c0:c1])
                    continue
                nc.sync.dma_start(xt[:, k * BT:(k + 1) * BT],
                                  x_in.ap()[k * 128:(k + 1) * 128, :])
                nc.sync.dma_start(wt[:, k * HE:(k + 1) * HE],
                                  w_in.ap()[k * 128:(k + 1) * 128, :])
            return xt, wt

        # --- V first: scoped pools free both PSUM and xtv SBUF space ---
        with tc.tile_pool(name="xtv", bufs=1) as xtv_p, \
             tc.tile_pool(name="psV", bufs=6, space="PSUM") as psV_p:
            xt, wt = load_xw(xv, wv, xtv_p, split_k0=True)
            nc.sync.dma_start(bvb[:], bv.ap().partition_broadcast(128))
            nc.sync.dma_start(bqc[:], bq.ap())
            nc.sync.dma_start(bkc[:], bk.ap())
            nc.sync.dma_start(boc[:], bo.ap().partition_broadcast(128))
            for b in range(B):
                for ti, (to, tsz) in enumerate(TT):
                    vbase = (b * 5 + ti) * VGW
                    bto = b * T + to
                    # ones blocks for all 16 heads of this tile (gpsimd)
                    og = sbVg[:tsz, vbase:vbase + VGW].rearrange(
                        "p (h c) -> p h c", c=128)
                    nc.gpsimd.memset(og[:, :, 0:E], 1.0)
                    # first chunk of the whole phase split to 256 so the
                    # opening matmul's weight dependency is only 66KB
                    chunks = ([(0, 256), (256, 256), (512, 512)]
                              if (b == 0 and ti == 0) else N512)
                    for (no, nsz) in chunks:
                        ps = psV_p.tile([128, nsz], F32, tag="psV", name="psV")
                        for k in range(KT):
                            nc.tensor.matmul(
                                ps[:tsz, :],
                                xt[:, k * BT + bto:k * BT + bto + tsz],
                                wt[:, k * HE + no:k * HE + no + nsz],
                                start=(k == 0), stop=(k == KT - 1))
                        # strided evict with b_V fold into [ones|V] groups
                        h0 = no // E
                        dst = sbVg[:tsz, vbase + h0 * 128:
                                   vbase + (h0 + nsz // E) * 128].rearrange(
                            "p (h c) -> p h c", c=128)[:, :, E:128]
                        nc.vector.tensor_add(
                            dst,
                            ps[:tsz, :].rearrange("p (h c) -> p h c", c=E),
                            bvb[:tsz, no:no + nsz].rearrange(
                                "p (h c) -> p h c", c=E))

        # --- Q/K inputs + W_O (early; wot reuses wtv's slot) ---
        xtq, wtq = load_xw(xq, wq, xt_p)
        xtk, wtk = load_xw(xk, wk, xt_p)
        wot = wt_p.tile([128, MT * D], BF16, tag="wt", name="wot")
        for hp in range(MT):
            nc.sync.dma_start(wot[:, hp * D:(hp + 1) * D],
                              wo.ap()[hp * 128:(hp + 1) * 128, :])

        # ========== A/B overlap region pools ==========
        ab = ExitStack()
        pp_p = ab.enter_context(tc.tile_pool(name="pp", bufs=1))
        rpf_p = ab.enter_context(tc.tile_pool(name="rpf", bufs=1))
        psS_p = ab.enter_context(tc.tile_pool(name="psS", bufs=2, space="PSUM"))
        pq1a_p = ab.enter_context(tc.tile_pool(name="pq1a", bufs=1, space="PSUM"))
        pq1b_p = ab.enter_context(tc.tile_pool(name="pq1b", bufs=1, space="PSUM"))
        psZ_p = ab.enter_context(tc.tile_pool(name="psZ", bufs=1, space="PSUM"))

        def emit_attn(b, hp):
            qb = hp * BT + b * T
            pzs = [psZ_p.tile([128, 512], F32, tag=f"psz{hd}", name=f"psz{hd}")
                   for hd in range(2)]
            pq1a = pq1a_p.tile([128, 512], F32, tag="pq1a", name="pq1a")
            pq1b = pq1b_p.tile([128, 512], F32, tag="pq1b", name="pq1b")
            pp = pp_p.tile([128, 5 * BT], BF16, tag="pp", name="pp")
            _last_pp[0] = pp
            # ---- S + exp(q0) per (kt, hd); S q1 into packed slots.
            #      AV-q0 for kt-2 interleaved so the PE has fill work
            #      while ACT runs the exps. ----
            def av_q0(kt, hd):
                ksz = TT[kt][1]
                h = 2 * hp + hd
                vcol = (b * 5 + kt) * VGW + h * 128
                nc.tensor.matmul(
                    pzs[hd][:, :],
                    sbVg[:ksz, vcol:vcol + 128],
                    pp[:ksz, kt * BT + hd * T:kt * BT + hd * T + Q0],
                    start=(kt == 0), stop=(kt == 4))

            ppv = pp[:, :].rearrange("p (k c) -> p k c", c=BT)

            def exp_q1(hd):
                # one strided exp per hd covering all five kt's q1 slots
                src = (pq1a if hd == 0 else pq1b)[:, 0:325]
                src = src.rearrange("p (k c) -> p k c", c=65)
                dst = ppv[:, :, hd * T + Q0:hd * T + Q0 + 65]
                nc.scalar.activation(dst, src, AF.Exp, scale=0.125)

            for kt, (ko, ksz) in enumerate(TT):
                for hd in range(2):
                    lo = hd * 64
                    ps = psS_p.tile([128, 512], F32, tag="psS", name="psS")
                    statK = sbK[lo:lo + 64, qb + ko:qb + ko + ksz]
                    nc.tensor.matmul(
                        ps[:ksz, :],
                        statK,
                        sbQ[lo:lo + 64, qb:qb + Q0],
                        start=True, stop=True, tile_position=(lo, 0))
                    q1dst = (pq1a if hd == 0 else pq1b)[
                        :ksz, kt * 65:kt * 65 + 65]
                    nc.tensor.matmul(
                        q1dst,
                        statK,
                        sbQ[lo:lo + 64, qb + Q0:qb + T],
                        start=True, stop=True, tile_position=(lo, 0))
                    nc.scalar.activation(
                        pp[:ksz, kt * BT + hd * T:kt * BT + hd * T + Q0],
                        ps[:ksz, :], AF.Exp, scale=0.125)
                    if kt == 4:
                        exp_q1(hd)
            # ---- AV q0 ----
            for kt in range(5):
                av_q0(kt, 0)
                av_q0(kt, 1)
            # ---- AV q1: single chains in pq1b spare cols, hd sequential ----
            for hd in range(2):
                for kt, (ko, ksz) in enumerate(TT):
                    h = 2 * hp + hd
                    vcol = (b * 5 + kt) * VGW + h * 128
                    nc.tensor.matmul(
                        pq1b[:, 325 + hd * 65:325 + hd * 65 + 65],
                        sbVg[:ksz, vcol:vcol + 128],
                        pp[:ksz,
                           kt * BT + hd * T + Q0:kt * BT + hd * T + T],
                        start=(kt == 0), stop=(kt == 4),
                        skip_group_check=True)
            # ---- normalize: recip of replicated denom rows, mul-evict ----
            for hd in range(2):
                lo = hd * 64
                rpf = rpf_p.tile([64, 584], F32, tag=f"rpf{hd}",
                                 name=f"rpf{hd}")
                nc.vector.reciprocal_approx_fast(
                    rpf[:, 0:Q0], pzs[hd][0:64, :])
                nc.vector.reciprocal_approx_fast(
                    rpf[:, Q0:T], pq1b[0:64, 325 + hd * 65:325 + hd * 65 + 65])
                nc.vector.tensor_mul(
                    zsl(b, hp, lo, 64, 0, Q0), pzs[hd][64:128, :],
                    rpf[:, 0:Q0])
                nc.vector.tensor_mul(
                    zsl(b, hp, lo, 64, Q0, Q1),
                    pq1b[64:128, 325 + hd * 65:325 + hd * 65 + 65],
                    rpf[:, Q0:T])

        # ---- interleaved Q/K projections + phase B ----
        with tc.tile_pool(name="psA", bufs=2, space="PSUM") as psA_p:
            for m in range(MT):
                for (xt, wt, b_col, dest) in ((xtq, wtq, bqc, sbQ),
                                              (xtk, wtk, bkc, sbK)):
                    for (no, nsz) in A_N:
                        ps = psA_p.tile([128, 386], F32, tag="psA", name="psA")
                        for k in range(KT):
                            nc.tensor.matmul(
                                ps[:, :nsz],
                                wt[:, k * HE + m * 128:k * HE + (m + 1) * 128],
                                xt[:, k * BT + no:k * BT + no + nsz],
                                start=(k == 0), stop=(k == KT - 1))
                        nc.vector.tensor_scalar_add(
                            dest[:, m * BT + no:m * BT + no + nsz],
                            ps[:, :nsz], b_col[:, m:m + 1])
                if m < MT - 1:
                    for b in range(B):
                        emit_attn(b, m)
        # psA closed: 2 banks free for psO

        # ================= Phase C: output projection =================
        sbO_p = ab.enter_context(tc.tile_pool(name="sbO", bufs=3))

        def emit_out(b, psO_p):
            for (mo, msz) in TT:
                for (no, nsz) in N512:
                    ps = psO_p.tile([128, 512], F32, tag="psO", name="psO")
                    for hp in range(MT):
                        nc.tensor.matmul(
                            ps[:msz, :],
                            zsl(b, hp, 0, 128, mo, msz),
                            wot[:, hp * D + no:hp * D + no + nsz],
                            start=(hp == 0), stop=(hp == MT - 1))
                    so = sbO_p.tile([128, 512], F32, tag="sbO", name="sbO")
                    if msz == 65:
                        for c0, c1 in ((0, 256), (256, 512)):
                            nc.vector.tensor_add(
                                so[:msz, c0:c1], ps[:msz, c0:c1],
                                boc[:msz, no + c0:no + c1])
                            nc.sync.dma_start(
                                out.ap()[b, mo:mo + msz,
                                         no + c0:no + c1],
                                so[:msz, c0:c1])
                    else:
                        nc.vector.tensor_add(so[:msz, :], ps[:msz, :],
                                             boc[:msz, no:no + nsz])
                        nc.sync.dma_start(
                            out.ap()[b, mo:mo + msz, no:no + nsz],
                            so[:msz, :])

        with tc.tile_pool(name="psO", bufs=2, space="PSUM") as psO_p:
            emit_attn(0, MT - 1)
            emit_out(0, psO_p)
            emit_attn(1, MT - 1)
            emit_out(1, psO_p)

        if _DEBUG_DUMPS is not None:
            for nm, t in (("dbg_sbQ", sbQ), ("dbg_sbK", sbK),
                          ("dbg_sbVg", sbVg), ("dbg_sbZ", sbZ),
                          ("dbg_pp", _last_pp[0])):
                d = nc.dram_tensor(nm, list(t.shape), BF16,
                                   kind="ExternalOutput")
                nc.sync.dma_start(d.ap(), t[:, :])
        ab.close()


_GRAPH = None


def _get_graph():
    global _GRAPH
    if _GRAPH is None:
        _GRAPH = build_graph()
    return _GRAPH


def kernel(query_input, key_input, value_input, W_Q, W_K, W_V, W_O,
           b_Q, b_K, b_V, b_O, _trace=False, _trace_kwargs=None):
    import ml_dtypes
    from concourse.bass_utils import run_bass_kernel_spmd

    nc = _get_graph()
    f = np.ascontiguousarray
    bf = ml_dtypes.bfloat16

    def xT(x, sl):
        x = np.asarray(x[sl], np.float32)
        return f(x.reshape(B * T, D).T.astype(bf))

    def wT(w):
        w = np.asarray(w, np.float32)
        return f(w.transpose(1, 0, 2).reshape(D, HE).astype(bf))

    def bcol(bx):
        bx = np.asarray(bx, np.float32).reshape(HE)
        return f(bx.reshape(MT, 128).T)

    wq_m, wk_m, wv_m = wT(W_Q), wT(W_K), wT(W_V)
    wo_m = f(np.asarray(W_O, np.float32).reshape(HE, D).astype(bf))
    bq_m, bk_m = bcol(b_Q), bcol(b_K)
    bv_m = f(np.asarray(b_V, np.float32).reshape(1, HE).astype(bf))
    bo_m = f(np.asarray(b_O, np.float32).reshape(1, D).astype(bf))
    in_maps = []
    for c in range(NCORES):
        sl = slice(2 * c, 2 * c + 2)
        in_maps.append({
            "query_input": xT(query_input, sl),
            "key_input": xT(key_input, sl),
            "value_input": xT(value_input, sl),
            "W_Q": wq_m,
            "W_K": wk_m,
            "W_V": wv_m,
            "W_O": wo_m,
            "b_Q": bq_m,
            "b_K": bk_m,
            "b_V": bv_m,
            "b_O": bo_m,
        })
    res = run_bass_kernel_spmd(nc, in_maps, core_ids=list(range(NCORES)),
                               trace=_trace, **(_trace_kwargs or {}))
    outp = np.concatenate([res.results[c]["out"] for c in range(NCORES)], axis=0)
    if _trace:
        kernel._last_result = res
    return outp


# revision 39
# speedup vs baseline: 1.0136x; 1.0136x over previous
"""Multi-head attention kernel for Trainium2, 8 NeuronCores, data-parallel over batch.

Problem: batch=16, pos=577, d_model=1024, n_heads=16, d_head=64, fp32.
Sharding: batch across 8 cores (2 batch items per core), no collectives.

v5 (final, ~268us vs 310us baseline): phase B restructured around big exp
instructions and merged AV matmuls.
  - q chunks (512, 65): one S stationary per (kt, hd) serves both chunks;
    exp instructions are [128,512] (q0) plus one batched strided exp per hd
    covering all five kt's 65-wide q1 slots, emitted as soon as kt4's S
    lands so the round tail is short.
  - AV stationary is [ones64 | V_h] (M=128, contiguous, FWL-eligible):
    PSUM rows 0:64 = softmax denominator replicated 64x, rows 64:128 = Z'.
    Normalization = 64-lane reciprocal_approx_fast (base-0 only!) +
    tensor-tensor multiply straight out of PSUM.
  - b_V folded into the V projection eviction (Z'/D = PV/D + b_V exactly);
    b_Q/b_K fused in Q/K evicts; b_O fused in the C-phase evict.
  - Tail: emit_attn(0,7) -> C(b=0) -> emit_attn(1,7) -> C(b=1) keeps the PE
    warm through the B->C transition.

PSUM banks (8): psS 2 (rotating S q0 staging), pq1a 1 (S-q1 slots hd0),
pq1b 1 (S-q1 slots hd1 + both AV-q1 chains, sequential), psZ 2 (AV-q0
accumulators per hd), psA 2 (Q/K projection staging).

has_written semantics learned the hard way: accumulation chains sharing a
bank must not interleave their start=True openers; sequential chains and
write-once groups are safe.
"""
import numpy as np

import concourse.bass as bass
import concourse.tile as tile
from concourse import bacc, mybir

F32 = mybir.dt.float32
BF16 = mybir.dt.bfloat16
AF = mybir.ActivationFunctionType

NCORES = 8
_DEBUG_DUMPS = None
B = 2            # batch per core
T = 577
D = 1024
H = 16
E = 64
HE = H * E       # 1024
BT = B * T       # 1154

KT = 8                                   # k-tiles over D
MT = 8                                   # m-tiles over HE (head pairs)
A_N = [(0, 386), (386, 384), (770, 384)]  # bt chunks for phase A
TT = [(0, 128), (128, 128), (256, 128), (384, 128), (512, 65)]  # tiles over T
N512 = [(0, 512), (512, 512)]            # 512-chunks over HE / D
VGW = H * 128                            # 2048: per (b,tile) [ones|V] groups
Q0 = 512                                 # q0 chunk width
Q1 = T - Q0                              # 65: q1 chunk width


def build_graph():
    nc = bacc.Bacc("TRN2", target_bir_lowering=False, debug=False,
                   num_devices=NCORES)

    xq = nc.dram_tensor("query_input", [D, BT], BF16, kind="ExternalInput")
    xk = nc.dram_tensor("key_input", [D, BT], BF16, kind="ExternalInput")
    xv = nc.dram_tensor("value_input", [D, BT], BF16, kind="ExternalInput")
    wq = nc.dram_tensor("W_Q", [D, HE], BF16, kind="ExternalInput")
    wk = nc.dram_tensor("W_K", [D, HE], BF16, kind="ExternalInput")
    wv = nc.dram_tensor("W_V", [D, HE], BF16, kind="ExternalInput")
    wo = nc.dram_tensor("W_O", [HE, D], BF16, kind="ExternalInput")
    bq = nc.dram_tensor("b_Q", [128, MT], F32, kind="ExternalInput")
    bk = nc.dram_tensor("b_K", [128, MT], F32, kind="ExternalInput")
    bv = nc.dram_tensor("b_V", [1, HE], BF16, kind="ExternalInput")
    bo = nc.dram_tensor("b_O", [1, D], BF16, kind="ExternalInput")
    out = nc.dram_tensor("out", [B, T, D], F32, kind="ExternalOutput")

    with tile.TileContext(nc) as tc:
        _body(nc, tc, xq, xk, xv, wq, wk, wv, wo, bq, bk, bv, bo, out)
    nc.compile()
    return nc


def _body(nc, tc, xq, xk, xv, wq, wk, wv, wo, bq, bk, bv, bo, out):
    from contextlib import ExitStack
    _last_pp = [None]
    est = ExitStack()
    with est:
        # ---- persistent pools; packed tiles ----
        sbQ_p = est.enter_context(tc.tile_pool(name="sbQ", bufs=1))
        sbK_p = est.enter_context(tc.tile_pool(name="sbK", bufs=1))
        sbVg_p = est.enter_context(tc.tile_pool(name="sbVg", bufs=1))
        sbZ_p = est.enter_context(tc.tile_pool(name="sbZ", bufs=1))
        xt_p = est.enter_context(tc.tile_pool(name="xt", bufs=2))
        wt_p = est.enter_context(tc.tile_pool(name="wt", bufs=3))
        const_p = est.enter_context(tc.tile_pool(name="const", bufs=1))

        bqc = const_p.tile([128, MT], F32, tag="bqc")
        bkc = const_p.tile([128, MT], F32, tag="bkc")
        bvb = const_p.tile([128, HE], BF16, tag="bvb")
        boc = const_p.tile([128, D], BF16, tag="boc")

        # packed persistent tiles (bf16)
        sbQ = sbQ_p.tile([128, MT * BT], BF16, tag="sbQ")     # [:, m*BT + bt]
        sbK = sbK_p.tile([128, MT * BT], BF16, tag="sbK")
        # per (b,tile): 16 head-groups of [ones64 | V_h 64]
        sbVg = sbVg_p.tile([128, 10 * VGW], BF16, tag="sbVg")
        sbZ = sbZ_p.tile([128, B * MT * T], BF16, tag="sbZ")  # [:, (b*MT+hp)*T + t]

        def zsl(b, hp, lo, sz, to, tsz):
            base = (b * MT + hp) * T
            return sbZ[lo:lo + sz, base + to:base + to + tsz]

        # ================= Phase A: projections =================
        def load_xw(x_in, w_in, pool, split_k0=False):
            xt = pool.tile([128, KT * BT], BF16, tag="xt", name="xt")
            wt = wt_p.tile([128, KT * HE], BF16, tag="wt", name="wt")
            for k in range(KT):
                if k == 0 and split_k0:
                    for c0, c1 in ((0, 128), (128, BT)):
                        nc.sync.dma_start(xt[:, c0:c1],
                                          x_in.ap()[0:128, c0:c1])
                    for c0, c1 in ((0, 512), (512, HE)):
                        nc.sync.dma_start(wt[:, c0:c1],
                                          w_in.ap()[0:128, c0:c1])
                    continue
                nc.sync.dma_start(xt[:, k * BT:(k + 1) * BT],
                                  x_in.ap()[k * 128:(k + 1) * 128, :])
                nc.sync.dma_start(wt[:, k * HE:(k + 1) * HE],
                                  w_in.ap()[k * 128:(k + 1) * 128, :])
            return xt, wt

        # --- V first: scoped pools free both PSUM and xtv SBUF space ---
        with tc.tile_pool(name="xtv", bufs=1) as xtv_p, \
             tc.tile_pool(name="psV", bufs=6, space="PSUM") as psV_p:
            xt, wt = load_xw(xv, wv, xtv_p)
            nc.sync.dma_start(bvb[:], bv.ap().partition_broadcast(128))
            nc.sync.dma_start(bqc[:], bq.ap())
            nc.sync.dma_start(bkc[:], bk.ap())
            nc.sync.dma_start(boc[:], bo.ap().partition_broadcast(128))
            for b in range(B):
                for ti, (to, tsz) in enumerate(TT):
                    vbase = (b * 5 + ti) * VGW
                    bto = b * T + to
                    # ones blocks for all 16 heads of this tile (gpsimd)
                    og = sbVg[:tsz, vbase:vbase + VGW].rearrange(
                        "p (h c) -> p h c", c=128)
                    nc.gpsimd.memset(og[:, :, 0:E], 1.0)
                    for (no, nsz) in N512:
                        ps = psV_p.tile([128, nsz], F32, tag="psV", name="psV")
                        for k in range(KT):
                            nc.tensor.matmul(
                                ps[:tsz, :],
                                xt[:, k * BT + bto:k * BT + bto + tsz],
                                wt[:, k * HE + no:k * HE + no + nsz],
                                start=(k == 0), stop=(k == KT - 1))
                        # strided evict with b_V fold into [ones|V] groups
                        h0 = no // E
                        dst = sbVg[:tsz, vbase + h0 * 128:
                                   vbase + (h0 + nsz // E) * 128].rearrange(
                            "p (h c) -> p h c", c=128)[:, :, E:128]
                        nc.vector.tensor_add(
                            dst,
                            ps[:tsz, :].rearrange("p (h c) -> p h c", c=E),
                            bvb[:tsz, no:no + nsz].rearrange(
                                "p (h c) -> p h c", c=E))

        # --- Q/K inputs + W_O (early; wot reuses wtv's slot) ---
        xtq, wtq = load_xw(xq, wq, xt_p)
        xtk, wtk = load_xw(xk, wk, xt_p)
        wot = wt_p.tile([128, MT * D], BF16, tag="wt", name="wot")
        for hp in range(MT):
            nc.sync.dma_start(wot[:, hp * D:(hp + 1) * D],
                              wo.ap()[hp * 128:(hp + 1) * 128, :])

        # ========== A/B overlap region pools ==========
        ab = ExitStack()
        pp_p = ab.enter_context(tc.tile_pool(name="pp", bufs=1))
        rpf_p = ab.enter_context(tc.tile_pool(name="rpf", bufs=1))
        psS_p = ab.enter_context(tc.tile_pool(name="psS", bufs=2, space="PSUM"))
        pq1a_p = ab.enter_context(tc.tile_pool(name="pq1a", bufs=1, space="PSUM"))
        pq1b_p = ab.enter_context(tc.tile_pool(name="pq1b", bufs=1, space="PSUM"))
        psZ_p = ab.enter_context(tc.tile_pool(name="psZ", bufs=1, space="PSUM"))

        def emit_attn(b, hp):
            qb = hp * BT + b * T
            pzs = [psZ_p.tile([128, 512], F32, tag=f"psz{hd}", name=f"psz{hd}")
                   for hd in range(2)]
            pq1a = pq1a_p.tile([128, 512], F32, tag="pq1a", name="pq1a")
            pq1b = pq1b_p.tile([128, 512], F32, tag="pq1b", name="pq1b")
            pp = pp_p.tile([128, 5 * BT], BF16, tag="pp", name="pp")
            _last_pp[0] = pp
            # ---- S + exp(q0) per (kt, hd); S q1 into packed slots.
            #      AV-q0 for kt-2 interleaved so the PE has fill work
            #      while ACT runs the exps. ----
            def av_q0(kt, hd):
                ksz = TT[kt][1]
                h = 2 * hp + hd
                vcol = (b * 5 + kt) * VGW + h * 128
                nc.tensor.matmul(
                    pzs[hd][:, :],
                    sbVg[:ksz, vcol:vcol + 128],
                    pp[:ksz, kt * BT + hd * T:kt * BT + hd * T + Q0],
                    start=(kt == 0), stop=(kt == 4))

            ppv = pp[:, :].rearrange("p (k c) -> p k c", c=BT)

            def exp_q1(hd):
                # one strided exp per hd covering all five kt's q1 slots
                src = (pq1a if hd == 0 else pq1b)[:, 0:325]
                src = src.rearrange("p (k c) -> p k c", c=65)
                dst = ppv[:, :, hd * T + Q0:hd * T + Q0 + 65]
                nc.scalar.activation(dst, src, AF.Exp, scale=0.125)

            for kt, (ko, ksz) in enumerate(TT):
                for hd in range(2):
                    lo = hd * 64
                    ps = psS_p.tile([128, 512], F32, tag="psS", name="psS")
                    statK = sbK[lo:lo + 64, qb + ko:qb + ko + ksz]
                    nc.tensor.matmul(
                        ps[:ksz, :],
                        statK,
                        sbQ[lo:lo + 64, qb:qb + Q0],
                        start=True, stop=True, tile_position=(lo, 0))
                    q1dst = (pq1a if hd == 0 else pq1b)[
                        :ksz, kt * 65:kt * 65 + 65]
                    nc.tensor.matmul(
                        q1dst,
                        statK,
                        sbQ[lo:lo + 64, qb + Q0:qb + T],
                        start=True, stop=True, tile_position=(lo, 0))
                    nc.scalar.activation(
                        pp[:ksz, kt * BT + hd * T:kt * BT + hd * T + Q0],
                        ps[:ksz, :], AF.Exp, scale=0.125)
                    if kt == 4:
                        exp_q1(hd)
            # ---- AV q0 ----
            for kt in range(5):
                av_q0(kt, 0)
                av_q0(kt, 1)
            # ---- AV q1: single chains in pq1b spare cols, hd sequential ----
            for hd in range(2):
                for kt, (ko, ksz) in enumerate(TT):
                    h = 2 * hp + hd
                    vcol = (b * 5 + kt) * VGW + h * 128
                    nc.tensor.matmul(
                        pq1b[:, 325 + hd * 65:325 + hd * 65 + 65],
                        sbVg[:ksz, vcol:vcol + 128],
                        pp[:ksz,
                           kt * BT + hd * T + Q0:kt * BT + hd * T + T],
                        start=(kt == 0), stop=(kt == 4),
                        skip_group_check=True)
            # ---- normalize: recip of replicated denom rows, mul-evict ----
            for hd in range(2):
                lo = hd * 64
                rpf = rpf_p.tile([64, 584], F32, tag=f"rpf{hd}",
                                 name=f"rpf{hd}")
                nc.vector.reciprocal_approx_fast(
                    rpf[:, 0:Q0], pzs[hd][0:64, :])
                nc.vector.reciprocal_approx_fast(
                    rpf[:, Q0:T], pq1b[0:64, 325 + hd * 65:325 + hd * 65 + 65])
                nc.vector.tensor_mul(
                    zsl(b, hp, lo, 64, 0, Q0), pzs[hd][64:128, :],
                    rpf[:, 0:Q0])
                nc.vector.tensor_mul(
                    zsl(b, hp, lo, 64, Q0, Q1),
                    pq1b[64:128, 325 + hd * 65:325 + hd * 65 + 65],
                    rpf[:, Q0:T])

        # ---- interleaved Q/K projections + phase B ----
        with tc.tile_pool(name="psA", bufs=2, space="PSUM") as psA_p:
            for m in range(MT):
                for (xt, wt, b_col, dest) in ((xtq, wtq, bqc, sbQ),
                                              (xtk, wtk, bkc, sbK)):
                    for (no, nsz) in A_N:
                        ps = psA_p.tile([128, 386], F32, tag="psA", name="psA")
                        for k in range(KT):
                            nc.tensor.matmul(
                                ps[:, :nsz],
                                wt[:, k * HE + m * 128:k * HE + (m + 1) * 128],
                                xt[:, k * BT + no:k * BT + no + nsz],
                                start=(k == 0), stop=(k == KT - 1))
                        nc.vector.tensor_scalar_add(
                            dest[:, m * BT + no:m * BT + no + nsz],
                            ps[:, :nsz], b_col[:, m:m + 1])
                if m < MT - 1:
                    for b in range(B):
                        emit_attn(b, m)
        # psA closed: 2 banks free for psO

        # ================= Phase C: output projection =================
        sbO_p = ab.enter_context(tc.tile_pool(name="sbO", bufs=3))

        def emit_out(b, psO_p):
            for (mo, msz) in TT:
                for (no, nsz) in N512:
                    ps = psO_p.tile([128, 512], F32, tag="psO", name="psO")
                    for hp in range(MT):
                        nc.tensor.matmul(
                            ps[:msz, :],
                            zsl(b, hp, 0, 128, mo, msz),
                            wot[:, hp * D + no:hp * D + no + nsz],
                            start=(hp == 0), stop=(hp == MT - 1))
                    so = sbO_p.tile([128, 512], F32, tag="sbO", name="sbO")
                    nc.vector.tensor_add(so[:msz, :], ps[:msz, :],
                                         boc[:msz, no:no + nsz])
                    nc.sync.dma_start(
                        out.ap()[b, mo:mo + msz, no:no + nsz], so[:msz, :])

        with tc.tile_pool(name="psO", bufs=2, space="PSUM") as psO_p:
            emit_attn(0, MT - 1)
            emit_out(0, psO_p)
            emit_attn(1, MT - 1)
            emit_out(1, psO_p)

        if _DEBUG_DUMPS is not None:
            for nm, t in (("dbg_sbQ", sbQ), ("dbg_sbK", sbK),
                          ("dbg_sbVg", sbVg), ("dbg_sbZ", sbZ),
                          ("dbg_pp", _last_pp[0])):
                d = nc.dram_tensor(nm, list(t.shape), BF16,
                                   kind="ExternalOutput")
                nc.sync.dma_start(d.ap(), t[:, :])
        ab.close()


_GRAPH = None


def _get_graph():
    global _GRAPH
    if _GRAPH is None:
        _GRAPH = build_graph()
    return _GRAPH


def kernel(query_input, key_input, value_input, W_Q, W_K, W_V, W_O,
           b_Q, b_K, b_V, b_O, _trace=False, _trace_kwargs=None):
    import ml_dtypes
    from concourse.bass_utils import run_bass_kernel_spmd

    nc = _get_graph()
    f = np.ascontiguousarray
    bf = ml_dtypes.bfloat16

    def xT(x, sl):
        x = np.asarray(x[sl], np.float32)
        return f(x.reshape(B * T, D).T.astype(bf))

    def wT(w):
        w = np.asarray(w, np.float32)
        return f(w.transpose(1, 0, 2).reshape(D, HE).astype(bf))

    def bcol(bx):
        bx = np.asarray(bx, np.float32).reshape(HE)
        return f(bx.reshape(MT, 128).T)

    wq_m, wk_m, wv_m = wT(W_Q), wT(W_K), wT(W_V)
    wo_m = f(np.asarray(W_O, np.float32).reshape(HE, D).astype(bf))
    bq_m, bk_m = bcol(b_Q), bcol(b_K)
    bv_m = f(np.asarray(b_V, np.float32).reshape(1, HE).astype(bf))
    bo_m = f(np.asarray(b_O, np.float32).reshape(1, D).astype(bf))
    in_maps = []
    for c in range(NCORES):
        sl = slice(2 * c, 2 * c + 2)
        in_maps.append({
            "query_input": xT(query_input, sl),
            "key_input": xT(key_input, sl),
            "value_input": xT(value_input, sl),
            "W_Q": wq_m,
            "W_K": wk_m,
            "W_V": wv_m,
            "W_O": wo_m,
            "b_Q": bq_m,
            "b_K": bk_m,
            "b_V": bv_m,
            "b_O": bo_m,
        })
    res = run_bass_kernel_spmd(nc, in_maps, core_ids=list(range(NCORES)),
                               trace=_trace, **(_trace_kwargs or {}))
    outp = np.concatenate([res.results[c]["out"] for c in range(NCORES)], axis=0)
    if _trace:
        kernel._last_result = res
    return outp


# revision 41
# speedup vs baseline: 1.0176x; 1.0039x over previous
"""Multi-head attention kernel for Trainium2, 8 NeuronCores, data-parallel over batch.

Problem: batch=16, pos=577, d_model=1024, n_heads=16, d_head=64, fp32.
Sharding: batch across 8 cores (2 batch items per core), no collectives.

v5 (final, ~268us vs 310us baseline): phase B restructured around big exp
instructions and merged AV matmuls.
  - q chunks (512, 65): one S stationary per (kt, hd) serves both chunks;
    exp instructions are [128,512] (q0) plus one batched strided exp per hd
    covering all five kt's 65-wide q1 slots, emitted as soon as kt4's S
    lands so the round tail is short.
  - AV stationary is [ones64 | V_h] (M=128, contiguous, FWL-eligible):
    PSUM rows 0:64 = softmax denominator replicated 64x, rows 64:128 = Z'.
    Normalization = 64-lane reciprocal_approx_fast (base-0 only!) +
    tensor-tensor multiply straight out of PSUM.
  - b_V folded into the V projection eviction (Z'/D = PV/D + b_V exactly);
    b_Q/b_K fused in Q/K evicts; b_O fused in the C-phase evict.
  - Tail: emit_attn(0,7) -> C(b=0) -> emit_attn(1,7) -> C(b=1) keeps the PE
    warm through the B->C transition.

PSUM banks (8): psS 2 (rotating S q0 staging), pq1a 1 (S-q1 slots hd0),
pq1b 1 (S-q1 slots hd1 + both AV-q1 chains, sequential), psZ 2 (AV-q0
accumulators per hd), psA 2 (Q/K projection staging).

has_written semantics learned the hard way: accumulation chains sharing a
bank must not interleave their start=True openers; sequential chains and
write-once groups are safe.
"""
import numpy as np

import concourse.bass as bass
import concourse.tile as tile
from concourse import bacc, mybir

F32 = mybir.dt.float32
BF16 = mybir.dt.bfloat16
AF = mybir.ActivationFunctionType

NCORES = 8
_DEBUG_DUMPS = None
B = 2            # batch per core
T = 577
D = 1024
H = 16
E = 64
HE = H * E       # 1024
BT = B * T       # 1154

KT = 8                                   # k-tiles over D
MT = 8                                   # m-tiles over HE (head pairs)
A_N = [(0, 386), (386, 384), (770, 384)]  # bt chunks for phase A
TT = [(0, 128), (128, 128), (256, 128), (384, 128), (512, 65)]  # tiles over T
N512 = [(0, 512), (512, 512)]            # 512-chunks over HE / D
VGW = H * 128                            # 2048: per (b,tile) [ones|V] groups
Q0 = 512                                 # q0 chunk width
Q1 = T - Q0                              # 65: q1 chunk width


def build_graph():
    nc = bacc.Bacc("TRN2", target_bir_lowering=False, debug=False,
                   num_devices=NCORES)

    xq = nc.dram_tensor("query_input", [D, BT], BF16, kind="ExternalInput")
    xk = nc.dram_tensor("key_input", [D, BT], BF16, kind="ExternalInput")
    xv = nc.dram_tensor("value_input", [D, BT], BF16, kind="ExternalInput")
    wq = nc.dram_tensor("W_Q", [D, HE], BF16, kind="ExternalInput")
    wk = nc.dram_tensor("W_K", [D, HE], BF16, kind="ExternalInput")
    wv = nc.dram_tensor("W_V", [D, HE], BF16, kind="ExternalInput")
    wo = nc.dram_tensor("W_O", [HE, D], BF16, kind="ExternalInput")
    bq = nc.dram_tensor("b_Q", [128, MT], F32, kind="ExternalInput")
    bk = nc.dram_tensor("b_K", [128, MT], F32, kind="ExternalInput")
    bv = nc.dram_tensor("b_V", [1, HE], BF16, kind="ExternalInput")
    bo = nc.dram_tensor("b_O", [1, D], BF16, kind="ExternalInput")
    out = nc.dram_tensor("out", [B, T, D], F32, kind="ExternalOutput")

    with tile.TileContext(nc) as tc:
        _body(nc, tc, xq, xk, xv, wq, wk, wv, wo, bq, bk, bv, bo, out)
    nc.compile()
    return nc


def _body(nc, tc, xq, xk, xv, wq, wk, wv, wo, bq, bk, bv, bo, out):
    from contextlib import ExitStack
    _last_pp = [None]
    est = ExitStack()
    with est:
        # ---- persistent pools; packed tiles ----
        sbQ_p = est.enter_context(tc.tile_pool(name="sbQ", bufs=1))
        sbK_p = est.enter_context(tc.tile_pool(name="sbK", bufs=1))
        sbVg_p = est.enter_context(tc.tile_pool(name="sbVg", bufs=1))
        sbZ_p = est.enter_context(tc.tile_pool(name="sbZ", bufs=1))
        xt_p = est.enter_context(tc.tile_pool(name="xt", bufs=2))
        wt_p = est.enter_context(tc.tile_pool(name="wt", bufs=3))
        const_p = est.enter_context(tc.tile_pool(name="const", bufs=1))

        bqc = const_p.tile([128, MT], F32, tag="bqc")
        bkc = const_p.tile([128, MT], F32, tag="bkc")
        bvb = const_p.tile([128, HE], BF16, tag="bvb")
        boc = const_p.tile([128, D], BF16, tag="boc")

        # packed persistent tiles (bf16)
        sbQ = sbQ_p.tile([128, MT * BT], BF16, tag="sbQ")     # [:, m*BT + bt]
        sbK = sbK_p.tile([128, MT * BT], BF16, tag="sbK")
        # per (b,tile): 16 head-groups of [ones64 | V_h 64]
        sbVg = sbVg_p.tile([128, 10 * VGW], BF16, tag="sbVg")
        sbZ = sbZ_p.tile([128, B * MT * T], BF16, tag="sbZ")  # [:, (b*MT+hp)*T + t]

        def zsl(b, hp, lo, sz, to, tsz):
            base = (b * MT + hp) * T
            return sbZ[lo:lo + sz, base + to:base + to + tsz]

        # ================= Phase A: projections =================
        def load_xw(x_in, w_in, pool, split_k0=False):
            xt = pool.tile([128, KT * BT], BF16, tag="xt", name="xt")
            wt = wt_p.tile([128, KT * HE], BF16, tag="wt", name="wt")
            for k in range(KT):
                if k == 0 and split_k0:
                    for c0, c1 in ((0, 128), (128, BT)):
                        nc.sync.dma_start(xt[:, c0:c1],
                                          x_in.ap()[0:128, c0:c1])
                    for c0, c1 in ((0, 512), (512, HE)):
                        nc.sync.dma_start(wt[:, c0:c1],
                                          w_in.ap()[0:128, c0:c1])
                    continue
                nc.sync.dma_start(xt[:, k * BT:(k + 1) * BT],
                                  x_in.ap()[k * 128:(k + 1) * 128, :])
                nc.sync.dma_start(wt[:, k * HE:(k + 1) * HE],
                                  w_in.ap()[k * 128:(k + 1) * 128, :])
            return xt, wt

        # --- V first: scoped pools free both PSUM and xtv SBUF space ---
        with tc.tile_pool(name="xtv", bufs=1) as xtv_p, \
             tc.tile_pool(name="psV", bufs=6, space="PSUM") as psV_p, \
             tc.tile_pool(name="warm", bufs=1, space="PSUM") as warm_p:
            xt, wt = load_xw(xv, wv, xtv_p)
            nc.sync.dma_start(bvb[:], bv.ap().partition_broadcast(128))
            nc.sync.dma_start(bqc[:], bq.ap())
            nc.sync.dma_start(bkc[:], bk.ap())
            nc.sync.dma_start(boc[:], bo.ap().partition_broadcast(128))
            warm = warm_p.tile([64, 64], F32, tag="warm", name="warm")
            for b in range(B):
                for ti, (to, tsz) in enumerate(TT):
                    vbase = (b * 5 + ti) * VGW
                    bto = b * T + to
                    # ones blocks for all 16 heads of this tile (gpsimd)
                    og = sbVg[:tsz, vbase:vbase + VGW].rearrange(
                        "p (h c) -> p h c", c=128)
                    nc.gpsimd.memset(og[:, :, 0:E], 1.0)
                    if b == 0 and ti == 0:
                        # HAM warm-up: keep the PE active through the input
                        # DMA wait so the first real matmuls run at 2.4GHz
                        for _ in range(60):
                            nc.tensor.matmul(
                                warm[:, :], sbVg[0:128, 0:E],
                                sbVg[0:128, 0:E], start=True, stop=True)
                    for (no, nsz) in N512:
                        ps = psV_p.tile([128, nsz], F32, tag="psV", name="psV")
                        for k in range(KT):
                            nc.tensor.matmul(
                                ps[:tsz, :],
                                xt[:, k * BT + bto:k * BT + bto + tsz],
                                wt[:, k * HE + no:k * HE + no + nsz],
                                start=(k == 0), stop=(k == KT - 1))
                        # strided evict with b_V fold into [ones|V] groups
                        h0 = no // E
                        dst = sbVg[:tsz, vbase + h0 * 128:
                                   vbase + (h0 + nsz // E) * 128].rearrange(
                            "p (h c) -> p h c", c=128)[:, :, E:128]
                        nc.vector.tensor_add(
                            dst,
                            ps[:tsz, :].rearrange("p (h c) -> p h c", c=E),
                            bvb[:tsz, no:no + nsz].rearrange(
                                "p (h c) -> p h c", c=E))

        # --- Q/K inputs + W_O (early; wot reuses wtv's slot) ---
        xtq, wtq = load_xw(xq, wq, xt_p)
        xtk, wtk = load_xw(xk, wk, xt_p)
        wot = wt_p.tile([128, MT * D], BF16, tag="wt", name="wot")
        for hp in range(MT):
            nc.sync.dma_start(wot[:, hp * D:(hp + 1) * D],
                              wo.ap()[hp * 128:(hp + 1) * 128, :])

        # ========== A/B overlap region pools ==========
        ab = ExitStack()
        pp_p = ab.enter_context(tc.tile_pool(name="pp", bufs=1))
        rpf_p = ab.enter_context(tc.tile_pool(name="rpf", bufs=1))
        psS_p = ab.enter_context(tc.tile_pool(name="psS", bufs=2, space="PSUM"))
        pq1a_p = ab.enter_context(tc.tile_pool(name="pq1a", bufs=1, space="PSUM"))
        pq1b_p = ab.enter_context(tc.tile_pool(name="pq1b", bufs=1, space="PSUM"))
        psZ_p = ab.enter_context(tc.tile_pool(name="psZ", bufs=1, space="PSUM"))

        def emit_attn(b, hp):
            qb = hp * BT + b * T
            pzs = [psZ_p.tile([128, 512], F32, tag=f"psz{hd}", name=f"psz{hd}")
                   for hd in range(2)]
            pq1a = pq1a_p.tile([128, 512], F32, tag="pq1a", name="pq1a")
            pq1b = pq1b_p.tile([128, 512], F32, tag="pq1b", name="pq1b")
            pp = pp_p.tile([128, 5 * BT], BF16, tag="pp", name="pp")
            _last_pp[0] = pp
            # ---- S + exp(q0) per (kt, hd); S q1 into packed slots.
            #      AV-q0 for kt-2 interleaved so the PE has fill work
            #      while ACT runs the exps. ----
            def av_q0(kt, hd):
                ksz = TT[kt][1]
                h = 2 * hp + hd
                vcol = (b * 5 + kt) * VGW + h * 128
                nc.tensor.matmul(
                    pzs[hd][:, :],
                    sbVg[:ksz, vcol:vcol + 128],
                    pp[:ksz, kt * BT + hd * T:kt * BT + hd * T + Q0],
                    start=(kt == 0), stop=(kt == 4))

            ppv = pp[:, :].rearrange("p (k c) -> p k c", c=BT)

            def exp_q1(hd):
                # one strided exp per hd covering all five kt's q1 slots
                src = (pq1a if hd == 0 else pq1b)[:, 0:325]
                src = src.rearrange("p (k c) -> p k c", c=65)
                dst = ppv[:, :, hd * T + Q0:hd * T + Q0 + 65]
                nc.scalar.activation(dst, src, AF.Exp, scale=0.125)

            for kt, (ko, ksz) in enumerate(TT):
                for hd in range(2):
                    lo = hd * 64
                    ps = psS_p.tile([128, 512], F32, tag="psS", name="psS")
                    statK = sbK[lo:lo + 64, qb + ko:qb + ko + ksz]
                    nc.tensor.matmul(
                        ps[:ksz, :],
                        statK,
                        sbQ[lo:lo + 64, qb:qb + Q0],
                        start=True, stop=True, tile_position=(lo, 0))
                    q1dst = (pq1a if hd == 0 else pq1b)[
                        :ksz, kt * 65:kt * 65 + 65]
                    nc.tensor.matmul(
                        q1dst,
                        statK,
                        sbQ[lo:lo + 64, qb + Q0:qb + T],
                        start=True, stop=True, tile_position=(lo, 0))
                    nc.scalar.activation(
                        pp[:ksz, kt * BT + hd * T:kt * BT + hd * T + Q0],
                        ps[:ksz, :], AF.Exp, scale=0.125)
                    if kt == 4:
                        exp_q1(hd)
            # ---- AV q0 ----
            for kt in range(5):
                av_q0(kt, 0)
                av_q0(kt, 1)
            # ---- AV q1: single chains in pq1b spare cols, hd sequential ----
            for hd in range(2):
                for kt, (ko, ksz) in enumerate(TT):
                    h = 2 * hp + hd
                    vcol = (b * 5 + kt) * VGW + h * 128
                    nc.tensor.matmul(
                        pq1b[:, 325 + hd * 65:325 + hd * 65 + 65],
                        sbVg[:ksz, vcol:vcol + 128],
                        pp[:ksz,
                           kt * BT + hd * T + Q0:kt * BT + hd * T + T],
                        start=(kt == 0), stop=(kt == 4),
                        skip_group_check=True)
            # ---- normalize: recip of replicated denom rows, mul-evict ----
            for hd in range(2):
                lo = hd * 64
                rpf = rpf_p.tile([64, 584], F32, tag=f"rpf{hd}",
                                 name=f"rpf{hd}")
                nc.vector.reciprocal_approx_fast(
                    rpf[:, 0:Q0], pzs[hd][0:64, :])
                nc.vector.reciprocal_approx_fast(
                    rpf[:, Q0:T], pq1b[0:64, 325 + hd * 65:325 + hd * 65 + 65])
                nc.vector.tensor_mul(
                    zsl(b, hp, lo, 64, 0, Q0), pzs[hd][64:128, :],
                    rpf[:, 0:Q0])
                nc.vector.tensor_mul(
                    zsl(b, hp, lo, 64, Q0, Q1),
                    pq1b[64:128, 325 + hd * 65:325 + hd * 65 + 65],
                    rpf[:, Q0:T])

        # ---- interleaved Q/K projections + phase B ----
        with tc.tile_pool(name="psA", bufs=2, space="PSUM") as psA_p:
            for m in range(MT):
                for (xt, wt, b_col, dest) in ((xtq, wtq, bqc, sbQ),
                                              (xtk, wtk, bkc, sbK)):
                    for (no, nsz) in A_N:
                        ps = psA_p.tile([128, 386], F32, tag="psA", name="psA")
                        for k in range(KT):
                            nc.tensor.matmul(
                                ps[:, :nsz],
                                wt[:, k * HE + m * 128:k * HE + (m + 1) * 128],
                                xt[:, k * BT + no:k * BT + no + nsz],
                                start=(k == 0), stop=(k == KT - 1))
                        nc.vector.tensor_scalar_add(
                            dest[:, m * BT + no:m * BT + no + nsz],
                            ps[:, :nsz], b_col[:, m:m + 1])
                if m < MT - 1:
                    for b in range(B):
                        emit_attn(b, m)
        # psA closed: 2 banks free for psO

        # ================= Phase C: output projection =================
        sbO_p = ab.enter_context(tc.tile_pool(name="sbO", bufs=3))

        def emit_out(b, psO_p):
            for (mo, msz) in TT:
                for (no, nsz) in N512:
                    ps = psO_p.tile([128, 512], F32, tag="psO", name="psO")
                    for hp in range(MT):
                        nc.tensor.matmul(
                            ps[:msz, :],
                            zsl(b, hp, 0, 128, mo, msz),
                            wot[:, hp * D + no:hp * D + no + nsz],
                            start=(hp == 0), stop=(hp == MT - 1))
                    so = sbO_p.tile([128, 512], F32, tag="sbO", name="sbO")
                    nc.vector.tensor_add(so[:msz, :], ps[:msz, :],
                                         boc[:msz, no:no + nsz])
                    nc.sync.dma_start(
                        out.ap()[b, mo:mo + msz, no:no + nsz], so[:msz, :])

        with tc.tile_pool(name="psO", bufs=2, space="PSUM") as psO_p:
            emit_attn(0, MT - 1)
            emit_out(0, psO_p)
            emit_attn(1, MT - 1)
            emit_out(1, psO_p)

        if _DEBUG_DUMPS is not None:
            for nm, t in (("dbg_sbQ", sbQ), ("dbg_sbK", sbK),
                          ("dbg_sbVg", sbVg), ("dbg_sbZ", sbZ),
                          ("dbg_pp", _last_pp[0])):
                d = nc.dram_tensor(nm, list(t.shape), BF16,
                                   kind="ExternalOutput")
                nc.sync.dma_start(d.ap(), t[:, :])
        ab.close()


_GRAPH = None


def _get_graph():
    global _GRAPH
    if _GRAPH is None:
        _GRAPH = build_graph()
    return _GRAPH


def kernel(query_input, key_input, value_input, W_Q, W_K, W_V, W_O,
           b_Q, b_K, b_V, b_O, _trace=False, _trace_kwargs=None):
    import ml_dtypes
    from concourse.bass_utils import run_bass_kernel_spmd

    nc = _get_graph()
    f = np.ascontiguousarray
    bf = ml_dtypes.bfloat16

    def xT(x, sl):
        x = np.asarray(x[sl], np.float32)
        return f(x.reshape(B * T, D).T.astype(bf))

    def wT(w):
        w = np.asarray(w, np.float32)
        return f(w.transpose(1, 0, 2).reshape(D, HE).astype(bf))

    def bcol(bx):
        bx = np.asarray(bx, np.float32).reshape(HE)
        return f(bx.reshape(MT, 128).T)

    wq_m, wk_m, wv_m = wT(W_Q), wT(W_K), wT(W_V)
    wo_m = f(np.asarray(W_O, np.float32).reshape(HE, D).astype(bf))
    bq_m, bk_m = bcol(b_Q), bcol(b_K)
    bv_m = f(np.asarray(b_V, np.float32).reshape(1, HE).astype(bf))
    bo_m = f(np.asarray(b_O, np.float32).reshape(1, D).astype(bf))
    in_maps = []
    for c in range(NCORES):
        sl = slice(2 * c, 2 * c + 2)
        in_maps.append({
            "query_input": xT(query_input, sl),
            "key_input": xT(key_input, sl),
            "value_input": xT(value_input, sl),
            "W_Q": wq_m,
            "W_K": wk_m,
            "W_V": wv_m,
            "W_O": wo_m,
            "b_Q": bq_m,
            "b_K": bk_m,
            "b_V": bv_m,
            "b_O": bo_m,
        })
    res = run_bass_kernel_spmd(nc, in_maps, core_ids=list(range(NCORES)),
                               trace=_trace, **(_trace_kwargs or {}))
    outp = np.concatenate([res.results[c]["out"] for c in range(NCORES)], axis=0)
    if _trace:
        kernel._last_result = res
    return outp


# revision 43
# speedup vs baseline: 1.0178x; 1.0002x over previous
"""Multi-head attention kernel for Trainium2, 8 NeuronCores, data-parallel over batch.

Problem: batch=16, pos=577, d_model=1024, n_heads=16, d_head=64, fp32.
Sharding: batch across 8 cores (2 batch items per core), no collectives.

v5 (final, ~268us vs 310us baseline): phase B restructured around big exp
instructions and merged AV matmuls.
  - q chunks (512, 65): one S stationary per (kt, hd) serves both chunks;
    exp instructions are [128,512] (q0) plus one batched strided exp per hd
    covering all five kt's 65-wide q1 slots, emitted as soon as kt4's S
    lands so the round tail is short.
  - AV stationary is [ones64 | V_h] (M=128, contiguous, FWL-eligible):
    PSUM rows 0:64 = softmax denominator replicated 64x, rows 64:128 = Z'.
    Normalization = 64-lane reciprocal_approx_fast (base-0 only!) +
    tensor-tensor multiply straight out of PSUM.
  - b_V folded into the V projection eviction (Z'/D = PV/D + b_V exactly);
    b_Q/b_K fused in Q/K evicts; b_O fused in the C-phase evict.
  - Tail: emit_attn(0,7) -> C(b=0) -> emit_attn(1,7) -> C(b=1) keeps the PE
    warm through the B->C transition.

PSUM banks (8): psS 2 (rotating S q0 staging), pq1a 1 (S-q1 slots hd0),
pq1b 1 (S-q1 slots hd1 + both AV-q1 chains, sequential), psZ 2 (AV-q0
accumulators per hd), psA 2 (Q/K projection staging).

has_written semantics learned the hard way: accumulation chains sharing a
bank must not interleave their start=True openers; sequential chains and
write-once groups are safe.
"""
import numpy as np

import concourse.bass as bass
import concourse.tile as tile
from concourse import bacc, mybir

F32 = mybir.dt.float32
BF16 = mybir.dt.bfloat16
AF = mybir.ActivationFunctionType

NCORES = 8
_DEBUG_DUMPS = None
B = 2            # batch per core
T = 577
D = 1024
H = 16
E = 64
HE = H * E       # 1024
BT = B * T       # 1154

KT = 8                                   # k-tiles over D
MT = 8                                   # m-tiles over HE (head pairs)
A_N = [(0, 386), (386, 384), (770, 384)]  # bt chunks for phase A
TT = [(0, 128), (128, 128), (256, 128), (384, 128), (512, 65)]  # tiles over T
N512 = [(0, 512), (512, 512)]            # 512-chunks over HE / D
VGW = H * 128                            # 2048: per (b,tile) [ones|V] groups
Q0 = 512                                 # q0 chunk width
Q1 = T - Q0                              # 65: q1 chunk width


def build_graph():
    nc = bacc.Bacc("TRN2", target_bir_lowering=False, debug=False,
                   num_devices=NCORES)

    xq = nc.dram_tensor("query_input", [D, BT], BF16, kind="ExternalInput")
    xk = nc.dram_tensor("key_input", [D, BT], BF16, kind="ExternalInput")
    xv = nc.dram_tensor("value_input", [D, BT], BF16, kind="ExternalInput")
    wq = nc.dram_tensor("W_Q", [D, HE], BF16, kind="ExternalInput")
    wk = nc.dram_tensor("W_K", [D, HE], BF16, kind="ExternalInput")
    wv = nc.dram_tensor("W_V", [D, HE], BF16, kind="ExternalInput")
    wo = nc.dram_tensor("W_O", [HE, D], BF16, kind="ExternalInput")
    bq = nc.dram_tensor("b_Q", [128, MT], F32, kind="ExternalInput")
    bk = nc.dram_tensor("b_K", [128, MT], F32, kind="ExternalInput")
    bv = nc.dram_tensor("b_V", [1, HE], BF16, kind="ExternalInput")
    bo = nc.dram_tensor("b_O", [1, D], BF16, kind="ExternalInput")
    out = nc.dram_tensor("out", [B, T, D], F32, kind="ExternalOutput")

    with tile.TileContext(nc) as tc:
        _body(nc, tc, xq, xk, xv, wq, wk, wv, wo, bq, bk, bv, bo, out)
    nc.compile()
    return nc


def _body(nc, tc, xq, xk, xv, wq, wk, wv, wo, bq, bk, bv, bo, out):
    from contextlib import ExitStack
    _last_pp = [None]
    est = ExitStack()
    with est:
        # ---- persistent pools; packed tiles ----
        sbQ_p = est.enter_context(tc.tile_pool(name="sbQ", bufs=1))
        sbK_p = est.enter_context(tc.tile_pool(name="sbK", bufs=1))
        sbVg_p = est.enter_context(tc.tile_pool(name="sbVg", bufs=1))
        sbZ_p = est.enter_context(tc.tile_pool(name="sbZ", bufs=1))
        xt_p = est.enter_context(tc.tile_pool(name="xt", bufs=2))
        wt_p = est.enter_context(tc.tile_pool(name="wt", bufs=3))
        const_p = est.enter_context(tc.tile_pool(name="const", bufs=1))

        bqc = const_p.tile([128, MT], F32, tag="bqc")
        bkc = const_p.tile([128, MT], F32, tag="bkc")
        bvb = const_p.tile([128, HE], BF16, tag="bvb")
        boc = const_p.tile([128, D], BF16, tag="boc")

        # packed persistent tiles (bf16)
        sbQ = sbQ_p.tile([128, MT * BT], BF16, tag="sbQ")     # [:, m*BT + bt]
        sbK = sbK_p.tile([128, MT * BT], BF16, tag="sbK")
        # per (b,tile): 16 head-groups of [ones64 | V_h 64]
        sbVg = sbVg_p.tile([128, 10 * VGW], BF16, tag="sbVg")
        sbZ = sbZ_p.tile([128, B * MT * T], BF16, tag="sbZ")  # [:, (b*MT+hp)*T + t]

        def zsl(b, hp, lo, sz, to, tsz):
            base = (b * MT + hp) * T
            return sbZ[lo:lo + sz, base + to:base + to + tsz]

        # ================= Phase A: projections =================
        def load_xw(x_in, w_in, pool, split_k0=False):
            xt = pool.tile([128, KT * BT], BF16, tag="xt", name="xt")
            wt = wt_p.tile([128, KT * HE], BF16, tag="wt", name="wt")
            for k in range(KT):
                if k == 0 and split_k0:
                    for c0, c1 in ((0, 128), (128, BT)):
                        nc.sync.dma_start(xt[:, c0:c1],
                                          x_in.ap()[0:128, c0:c1])
                    for c0, c1 in ((0, 512), (512, HE)):
                        nc.sync.dma_start(wt[:, c0:c1],
                                          w_in.ap()[0:128, c0:c1])
                    continue
                nc.sync.dma_start(xt[:, k * BT:(k + 1) * BT],
                                  x_in.ap()[k * 128:(k + 1) * 128, :])
                nc.sync.dma_start(wt[:, k * HE:(k + 1) * HE],
                                  w_in.ap()[k * 128:(k + 1) * 128, :])
            return xt, wt

        # --- V first: scoped pools free both PSUM and xtv SBUF space ---
        with tc.tile_pool(name="xtv", bufs=1) as xtv_p, \
             tc.tile_pool(name="psV", bufs=6, space="PSUM") as psV_p, \
             tc.tile_pool(name="warm", bufs=1, space="PSUM") as warm_p:
            warm = warm_p.tile([64, 64], F32, tag="warm", name="warm")
            cap = nc.const_aps.aps[(mybir.dt.bfloat16, 1.0)]
            for _ in range(150):
                # HAM warm-up: dependency-free PE activity through the input
                # DMA wait so the first real matmuls run at 2.4GHz
                nc.tensor.matmul(warm[0:1, 0:1], cap, cap,
                                 start=True, stop=True)
            xt, wt = load_xw(xv, wv, xtv_p)
            nc.sync.dma_start(bvb[:], bv.ap().partition_broadcast(128))
            nc.sync.dma_start(bqc[:], bq.ap())
            nc.sync.dma_start(bkc[:], bk.ap())
            nc.sync.dma_start(boc[:], bo.ap().partition_broadcast(128))
            for b in range(B):
                for ti, (to, tsz) in enumerate(TT):
                    vbase = (b * 5 + ti) * VGW
                    bto = b * T + to
                    # ones blocks for all 16 heads of this tile (gpsimd)
                    og = sbVg[:tsz, vbase:vbase + VGW].rearrange(
                        "p (h c) -> p h c", c=128)
                    nc.gpsimd.memset(og[:, :, 0:E], 1.0)

                    for (no, nsz) in N512:
                        ps = psV_p.tile([128, nsz], F32, tag="psV", name="psV")
                        for k in range(KT):
                            nc.tensor.matmul(
                                ps[:tsz, :],
                                xt[:, k * BT + bto:k * BT + bto + tsz],
                                wt[:, k * HE + no:k * HE + no + nsz],
                                start=(k == 0), stop=(k == KT - 1))
                        # strided evict with b_V fold into [ones|V] groups
                        h0 = no // E
                        dst = sbVg[:tsz, vbase + h0 * 128:
                                   vbase + (h0 + nsz // E) * 128].rearrange(
                            "p (h c) -> p h c", c=128)[:, :, E:128]
                        nc.vector.tensor_add(
                            dst,
                            ps[:tsz, :].rearrange("p (h c) -> p h c", c=E),
                            bvb[:tsz, no:no + nsz].rearrange(
                                "p (h c) -> p h c", c=E))

        # --- Q/K inputs + W_O (early; wot reuses wtv's slot) ---
        xtq, wtq = load_xw(xq, wq, xt_p)
        xtk, wtk = load_xw(xk, wk, xt_p)
        wot = wt_p.tile([128, MT * D], BF16, tag="wt", name="wot")
        for hp in range(MT):
            nc.sync.dma_start(wot[:, hp * D:(hp + 1) * D],
                              wo.ap()[hp * 128:(hp + 1) * 128, :])

        # ========== A/B overlap region pools ==========
        ab = ExitStack()
        pp_p = ab.enter_context(tc.tile_pool(name="pp", bufs=1))
        rpf_p = ab.enter_context(tc.tile_pool(name="rpf", bufs=1))
        psS_p = ab.enter_context(tc.tile_pool(name="psS", bufs=2, space="PSUM"))
        pq1a_p = ab.enter_context(tc.tile_pool(name="pq1a", bufs=1, space="PSUM"))
        pq1b_p = ab.enter_context(tc.tile_pool(name="pq1b", bufs=1, space="PSUM"))
        psZ_p = ab.enter_context(tc.tile_pool(name="psZ", bufs=1, space="PSUM"))

        def emit_attn(b, hp):
            qb = hp * BT + b * T
            pzs = [psZ_p.tile([128, 512], F32, tag=f"psz{hd}", name=f"psz{hd}")
                   for hd in range(2)]
            pq1a = pq1a_p.tile([128, 512], F32, tag="pq1a", name="pq1a")
            pq1b = pq1b_p.tile([128, 512], F32, tag="pq1b", name="pq1b")
            pp = pp_p.tile([128, 5 * BT], BF16, tag="pp", name="pp")
            _last_pp[0] = pp
            # ---- S + exp(q0) per (kt, hd); S q1 into packed slots.
            #      AV-q0 for kt-2 interleaved so the PE has fill work
            #      while ACT runs the exps. ----
            def av_q0(kt, hd):
                ksz = TT[kt][1]
                h = 2 * hp + hd
                vcol = (b * 5 + kt) * VGW + h * 128
                nc.tensor.matmul(
                    pzs[hd][:, :],
                    sbVg[:ksz, vcol:vcol + 128],
                    pp[:ksz, kt * BT + hd * T:kt * BT + hd * T + Q0],
                    start=(kt == 0), stop=(kt == 4))

            ppv = pp[:, :].rearrange("p (k c) -> p k c", c=BT)

            def exp_q1(hd):
                # one strided exp per hd covering all five kt's q1 slots
                src = (pq1a if hd == 0 else pq1b)[:, 0:325]
                src = src.rearrange("p (k c) -> p k c", c=65)
                dst = ppv[:, :, hd * T + Q0:hd * T + Q0 + 65]
                nc.scalar.activation(dst, src, AF.Exp, scale=0.125)

            for kt, (ko, ksz) in enumerate(TT):
                for hd in range(2):
                    lo = hd * 64
                    ps = psS_p.tile([128, 512], F32, tag="psS", name="psS")
                    statK = sbK[lo:lo + 64, qb + ko:qb + ko + ksz]
                    nc.tensor.matmul(
                        ps[:ksz, :],
                        statK,
                        sbQ[lo:lo + 64, qb:qb + Q0],
                        start=True, stop=True, tile_position=(lo, 0))
                    q1dst = (pq1a if hd == 0 else pq1b)[
                        :ksz, kt * 65:kt * 65 + 65]
                    nc.tensor.matmul(
                        q1dst,
                        statK,
                        sbQ[lo:lo + 64, qb + Q0:qb + T],
                        start=True, stop=True, tile_position=(lo, 0))
                    nc.scalar.activation(
                        pp[:ksz, kt * BT + hd * T:kt * BT + hd * T + Q0],
                        ps[:ksz, :], AF.Exp, scale=0.125)
                    if kt == 4:
                        exp_q1(hd)
            # ---- AV q0 ----
            for kt in range(5):
                av_q0(kt, 0)
                av_q0(kt, 1)
            # ---- AV q1: single chains in pq1b spare cols, hd sequential ----
            for hd in range(2):
                for kt, (ko, ksz) in enumerate(TT):
                    h = 2 * hp + hd
                    vcol = (b * 5 + kt) * VGW + h * 128
                    nc.tensor.matmul(
                        pq1b[:, 325 + hd * 65:325 + hd * 65 + 65],
                        sbVg[:ksz, vcol:vcol + 128],
                        pp[:ksz,
                           kt * BT + hd * T + Q0:kt * BT + hd * T + T],
                        start=(kt == 0), stop=(kt == 4),
                        skip_group_check=True)
            # ---- normalize: recip of replicated denom rows, mul-evict ----
            for hd in range(2):
                lo = hd * 64
                rpf = rpf_p.tile([64, 584], F32, tag=f"rpf{hd}",
                                 name=f"rpf{hd}")
                nc.vector.reciprocal_approx_fast(
                    rpf[:, 0:Q0], pzs[hd][0:64, :])
                nc.vector.reciprocal_approx_fast(
                    rpf[:, Q0:T], pq1b[0:64, 325 + hd * 65:325 + hd * 65 + 65])
                nc.vector.tensor_mul(
                    zsl(b, hp, lo, 64, 0, Q0), pzs[hd][64:128, :],
                    rpf[:, 0:Q0])
                nc.vector.tensor_mul(
                    zsl(b, hp, lo, 64, Q0, Q1),
                    pq1b[64:128, 325 + hd * 65:325 + hd * 65 + 65],
                    rpf[:, Q0:T])

        # ---- interleaved Q/K projections + phase B ----
        with tc.tile_pool(name="psA", bufs=2, space="PSUM") as psA_p:
            for m in range(MT):
                for (xt, wt, b_col, dest) in ((xtq, wtq, bqc, sbQ),
                                              (xtk, wtk, bkc, sbK)):
                    for (no, nsz) in A_N:
                        ps = psA_p.tile([128, 386], F32, tag="psA", name="psA")
                        for k in range(KT):
                            nc.tensor.matmul(
                                ps[:, :nsz],
                                wt[:, k * HE + m * 128:k * HE + (m + 1) * 128],
                                xt[:, k * BT + no:k * BT + no + nsz],
                                start=(k == 0), stop=(k == KT - 1))
                        nc.vector.tensor_scalar_add(
                            dest[:, m * BT + no:m * BT + no + nsz],
                            ps[:, :nsz], b_col[:, m:m + 1])
                if m < MT - 1:
                    for b in range(B):
                        emit_attn(b, m)
        # psA closed: 2 banks free for psO

        # ================= Phase C: output projection =================
        sbO_p = ab.enter_context(tc.tile_pool(name="sbO", bufs=3))

        def emit_out(b, psO_p):
            for (mo, msz) in TT:
                for (no, nsz) in N512:
                    ps = psO_p.tile([128, 512], F32, tag="psO", name="psO")
                    for hp in range(MT):
                        nc.tensor.matmul(
                            ps[:msz, :],
                            zsl(b, hp, 0, 128, mo, msz),
                            wot[:, hp * D + no:hp * D + no + nsz],
                            start=(hp == 0), stop=(hp == MT - 1))
                    so = sbO_p.tile([128, 512], F32, tag="sbO", name="sbO")
                    nc.vector.tensor_add(so[:msz, :], ps[:msz, :],
                                         boc[:msz, no:no + nsz])
                    nc.sync.dma_start(
                        out.ap()[b, mo:mo + msz, no:no + nsz], so[:msz, :])

        with tc.tile_pool(name="psO", bufs=2, space="PSUM") as psO_p:
            emit_attn(0, MT - 1)
            emit_out(0, psO_p)
            emit_attn(1, MT - 1)
            emit_out(1, psO_p)

        if _DEBUG_DUMPS is not None:
            for nm, t in (("dbg_sbQ", sbQ), ("dbg_sbK", sbK),
                          ("dbg_sbVg", sbVg), ("dbg_sbZ", sbZ),
                          ("dbg_pp", _last_pp[0])):
                d = nc.dram_tensor(nm, list(t.shape), BF16,
                                   kind="ExternalOutput")
                nc.sync.dma_start(d.ap(), t[:, :])
        ab.close()


_GRAPH = None


def _get_graph():
    global _GRAPH
    if _GRAPH is None:
        _GRAPH = build_graph()
    return _GRAPH


def kernel(query_input, key_input, value_input, W_Q, W_K, W_V, W_O,
           b_Q, b_K, b_V, b_O, _trace=False, _trace_kwargs=None):
    import ml_dtypes
    from concourse.bass_utils import run_bass_kernel_spmd

    nc = _get_graph()
    f = np.ascontiguousarray
    bf = ml_dtypes.bfloat16

    def xT(x, sl):
        x = np.asarray(x[sl], np.float32)
        return f(x.reshape(B * T, D).T.astype(bf))

    def wT(w):
        w = np.asarray(w, np.float32)
        return f(w.transpose(1, 0, 2).reshape(D, HE).astype(bf))

    def bcol(bx):
        bx = np.asarray(bx, np.float32).reshape(HE)
        return f(bx.reshape(MT, 128).T)

    wq_m, wk_m, wv_m = wT(W_Q), wT(W_K), wT(W_V)
    wo_m = f(np.asarray(W_O, np.float32).reshape(HE, D).astype(bf))
    bq_m, bk_m = bcol(b_Q), bcol(b_K)
    bv_m = f(np.asarray(b_V, np.float32).reshape(1, HE).astype(bf))
    bo_m = f(np.asarray(b_O, np.float32).reshape(1, D).astype(bf))
    in_maps = []
    for c in range(NCORES):
        sl = slice(2 * c, 2 * c + 2)
        in_maps.append({
            "query_input": xT(query_input, sl),
            "key_input": xT(key_input, sl),
            "value_input": xT(value_input, sl),
            "W_Q": wq_m,
            "W_K": wk_m,
            "W_V": wv_m,
            "W_O": wo_m,
            "b_Q": bq_m,
            "b_K": bk_m,
            "b_V": bv_m,
            "b_O": bo_m,
        })
    res = run_bass_kernel_spmd(nc, in_maps, core_ids=list(range(NCORES)),
                               trace=_trace, **(_trace_kwargs or {}))
    outp = np.concatenate([res.results[c]["out"] for c in range(NCORES)], axis=0)
    if _trace:
        kernel._last_result = res
    return outp
